# revision 37
# baseline (speedup 1.0000x reference)
"""Batch-sharded scaled-dot-product attention (with weights output) on 8 TRN2 NeuronCores.

Problem: B=16, LQ=LK=2048, D=512, f32 inputs. Returns (weighted_sum, weights)
exactly like the reference (softmax over keys, bool mask with True=masked,
masked weights exactly 0).

Sharding: batch dim 16 -> 2 batches per core, fully data-parallel (no
collectives).

Per-core dataflow (no DMA transposes of E; S^T computed on the PE):
  - Q,K,V loaded f32 (plain SWDGE), cast to bf16 on DVE.
  - Q^T,K^T via xbar DMA transpose, 4 big calls each per batch, Sync queue
    only (xbar state is global; two queues corrupt).
  - S   = Q^T.T @ K^T  (+ ones x bias_row matmul adds -1e9 to masked cols)
    -> exp with accum_out rowsums -> recip -> W=E*recip -> one store/q-tile.
  - S^T = K^T.T @ Q^T (same operands swapped); exp' applies the mask via
    ACT per-partition bias (-1e9 on masked key rows) -> E^T bf16 tiles.
    Query dim processed in 2 halves to bound SBUF.
  - O   = sum_i E^T_i.T @ V_i, scaled by recip during the PSUM->SBUF copy.
"""

import math
import numpy as np

from concourse import mybir, bacc, tile
from concourse.bass_utils import run_bass_kernel_spmd

F32 = mybir.dt.float32
BF16 = mybir.dt.bfloat16
AF = mybir.ActivationFunctionType

B, LQ, LK, D = 16, 2048, 2048, 512
NCORES = 8
BPC = B // NCORES  # batches per core
P = 128
DC = D // P        # 4 contraction chunks
NKC = LK // P      # 16 key chunks
NQT = LQ // P      # 16 query tiles
SEG = 512
NSEG = LK // SEG   # 4 key segments
NQH = NQT // 2     # q-tiles per half
SCALE = 1.0 / math.sqrt(D)
MASK_BIAS = -1.0e9

_CACHE = {}


def _build_program():
    from contextlib import ExitStack

    nc = bacc.Bacc("TRN2", target_bir_lowering=False, debug=False, num_devices=NCORES)
    q_d = nc.dram_tensor("query", [BPC, LQ, D], F32, kind="ExternalInput").ap()
    k_d = nc.dram_tensor("keys", [BPC, LK, D], F32, kind="ExternalInput").ap()
    v_d = nc.dram_tensor("values", [BPC, LK, D], F32, kind="ExternalInput").ap()
    mb_d = nc.dram_tensor("maskbias", [BPC, LK], F32, kind="ExternalInput").ap()
    o_d = nc.dram_tensor("out", [BPC, LQ, D], F32, kind="ExternalOutput").ap()
    w_d = nc.dram_tensor("weights", [BPC, LQ, LK], F32, kind="ExternalOutput").ap()

    with tile.TileContext(nc) as tc, ExitStack() as ctx:
        const_pool = ctx.enter_context(tc.tile_pool(name="const", bufs=1))
        bias_pool = ctx.enter_context(tc.tile_pool(name="bias", bufs=2))
        mbias_pool = ctx.enter_context(tc.tile_pool(name="mbias", bufs=2))
        stage_pool = ctx.enter_context(tc.tile_pool(name="stage", bufs=3))
        kload_pool = ctx.enter_context(tc.tile_pool(name="kload", bufs=2))
        qload_pool = ctx.enter_context(tc.tile_pool(name="qload", bufs=2))
        v_pool = ctx.enter_context(tc.tile_pool(name="v", bufs=8))
        kt_pool = ctx.enter_context(tc.tile_pool(name="kt", bufs=2))
        qt_pool = ctx.enter_context(tc.tile_pool(name="qt", bufs=2))
        e_pool = ctx.enter_context(tc.tile_pool(name="e", bufs=4))
        et_pool = ctx.enter_context(tc.tile_pool(name="et", bufs=16))
        w_pool = ctx.enter_context(tc.tile_pool(name="w", bufs=2))
        o_pool = ctx.enter_context(tc.tile_pool(name="o", bufs=2))
        stat_pool = ctx.enter_context(tc.tile_pool(name="stat", bufs=4))
        recip_pool = ctx.enter_context(tc.tile_pool(name="recip", bufs=20))
        s_psum = ctx.enter_context(tc.tile_pool(name="spsum", bufs=2, space="PSUM"))
        s2_psum = ctx.enter_context(tc.tile_pool(name="s2psum", bufs=2, space="PSUM"))
        o_psum = ctx.enter_context(tc.tile_pool(name="opsum", bufs=2, space="PSUM"))

        ones = const_pool.tile([1, P], BF16)
        nc.gpsimd.memset(ones[:], 1.0)

        for b in range(BPC):
            # ---------------- per-batch prep ----------------
            bias_row = bias_pool.tile([1, LK], BF16)
            nc.gpsimd.dma_start(bias_row[:], mb_d[b : b + 1, :])  # f32 -> bf16

            # mask bias as per-partition columns: mbias[:, i] = bias_row[128i..]
            pm = s2_psum.tile([P, SEG], F32, name="s2")
            for i in range(NKC):
                nc.tensor.matmul(
                    pm[:, i : i + 1],
                    bias_row[:, i * P : (i + 1) * P],
                    ones[:, 0:1],
                    start=True,
                    stop=True,
                )
            mbias = mbias_pool.tile([P, NKC], F32)
            nc.vector.tensor_copy(mbias[:], pm[:, :NKC])

            k_r = k_d[b].rearrange("(n p) d -> p n d", p=P)  # [128, NKC, D]
            v_r = v_d[b].rearrange("(n p) d -> p n d", p=P)
            q_r = q_d[b].rearrange("(n p) d -> p n d", p=P)

            # kt/qt layout: [128(d-local), chunk i/j, d-chunk c, 128]
            kt = kt_pool.tile([P, NKC, DC, P], BF16, name="kt")
            qt = qt_pool.tile([P, NQT, DC, P], BF16, name="qt")
            vb = []
            if b == 0:
                # Cold start: nothing to hide the prep under. K first (the
                # first q-tile consumes all of K^T), Q concurrently on the
                # scalar HWDGE queue (idle this early; per-queue streams are
                # independent), V last (mm2 needs it much later and the
                # 16-deep et pool rides out the wait).
                for g in range(4):
                    ks = stage_pool.tile([P, 4, D], F32, name="stage")
                    nc.gpsimd.dma_start(ks[:], k_r[:, 4 * g : 4 * g + 4, :])
                    kg = kload_pool.tile([P, 4, D], BF16, name="kg")
                    nc.vector.tensor_copy(kg[:], ks[:])
                    nc.sync.dma_start(
                        kt[:, 4 * g : 4 * g + 4, :, :], kg[:], transpose=True
                    )
                    qs = stage_pool.tile([P, 4, D], F32, name="stage")
                    nc.scalar.dma_start(qs[:], q_r[:, 4 * g : 4 * g + 4, :])
                    qg = qload_pool.tile([P, 4, D], BF16, name="qg")
                    nc.vector.tensor_copy(qg[:], qs[:])
                    nc.sync.dma_start(
                        qt[:, 4 * g : 4 * g + 4, :, :], qg[:], transpose=True
                    )
                for g in range(4):
                    vs = stage_pool.tile([P, 4, D], F32, name="stage")
                    nc.gpsimd.dma_start(vs[:], v_r[:, 4 * g : 4 * g + 4, :])
                    vt = v_pool.tile([P, 4, D], BF16, name="vt")
                    nc.vector.tensor_copy(vt[:], vs[:])
                    vb.append(vt)
            else:
                for g in range(4):
                    ks = stage_pool.tile([P, 4, D], F32, name="stage")
                    nc.gpsimd.dma_start(ks[:], k_r[:, 4 * g : 4 * g + 4, :])
                    kg = kload_pool.tile([P, 4, D], BF16, name="kg")
                    nc.vector.tensor_copy(kg[:], ks[:])
                    nc.sync.dma_start(
                        kt[:, 4 * g : 4 * g + 4, :, :], kg[:], transpose=True
                    )
                    qs = stage_pool.tile([P, 4, D], F32, name="stage")
                    nc.gpsimd.dma_start(qs[:], q_r[:, 4 * g : 4 * g + 4, :])
                    qg = qload_pool.tile([P, 4, D], BF16, name="qg")
                    nc.vector.tensor_copy(qg[:], qs[:])
                    nc.sync.dma_start(
                        qt[:, 4 * g : 4 * g + 4, :, :], qg[:], transpose=True
                    )
                    vs = stage_pool.tile([P, 4, D], F32, name="stage")
                    nc.gpsimd.dma_start(vs[:], v_r[:, 4 * g : 4 * g + 4, :])
                    vt = v_pool.tile([P, 4, D], BF16, name="vt")
                    nc.vector.tensor_copy(vt[:], vs[:])
                    vb.append(vt)

            # ---------------- halves of the query dim ----------------
            for h in range(2):
                jbase = h * NQH
                # -- W side: S, exp, rowsums, W out --
                for jj in range(NQH):
                    j = jbase + jj
                    rowsums = stat_pool.tile([P, 2], F32, name="rowsums")
                    epairs = []
                    for pr in range(2):  # pairs of key segments -> one exp
                        sp = s_psum.tile([P, 2 * SEG], F32, name="sp")
                        for s2i in range(2):
                            s = 2 * pr + s2i
                            sph = sp[:, s2i * SEG : (s2i + 1) * SEG]
                            for c in range(DC):
                                nc.tensor.matmul(
                                    sph,
                                    qt[:, j, c, :],
                                    kt[:, 4 * s : 4 * s + 4, c, :],
                                    start=(c == 0),
                                    stop=False,
                                )
                            nc.tensor.matmul(
                                sph,
                                ones[:],
                                bias_row[:, s * SEG : (s + 1) * SEG],
                                start=False,
                                stop=True,
                            )
                        e = e_pool.tile([P, 2 * SEG], BF16, name="e")
                        nc.scalar.activation(
                            e[:],
                            sp[:],
                            AF.Exp,
                            scale=SCALE,
                            accum_out=rowsums[:, pr : pr + 1],
                        )
                        epairs.append(e)

                    sum_all = stat_pool.tile([P, 1], F32, name="sumall")
                    nc.vector.reduce_sum(
                        sum_all[:], rowsums[:], axis=mybir.AxisListType.X
                    )
                    recip = recip_pool.tile([P, 1], F32, name="recip")
                    nc.vector.reciprocal(recip[:], sum_all[:])
                    _CACHE.setdefault("recips", {})[(b, j)] = recip

                    for wh in range(2):
                        wf = w_pool.tile([P, LK // 2], F32, name="wf")
                        nc.vector.tensor_scalar_mul(
                            wf[:], epairs[wh][:], recip[:]
                        )
                        nc.scalar.dma_start(
                            w_d[
                                b,
                                j * P : (j + 1) * P,
                                wh * (LK // 2) : (wh + 1) * (LK // 2),
                            ],
                            wf[:],
                        )

                # -- S^T side: E^T tiles for this half --
                ets = []
                for i in range(NKC):
                    et = et_pool.tile([P, NQH * P], BF16, name="et")
                    for t in range(2):
                        s2 = s2_psum.tile([P, SEG], F32, name="s2")
                        tq = 2 * h + t
                        for c in range(DC):
                            nc.tensor.matmul(
                                s2[:],
                                kt[:, i, c, :],
                                qt[:, 4 * tq : 4 * tq + 4, c, :],
                                start=(c == 0),
                                stop=(c == DC - 1),
                            )
                        nc.scalar.activation(
                            et[:, t * SEG : (t + 1) * SEG],
                            s2[:],
                            AF.Exp,
                            scale=SCALE,
                            bias=mbias[:, i : i + 1],
                        )
                    ets.append(et)

                # -- O side: mm2 + scale + store --
                for jj in range(NQH):
                    j = jbase + jj
                    op = o_psum.tile([P, D], F32, name="op")
                    for i in range(NKC):
                        nc.tensor.matmul(
                            op[:],
                            ets[i][:, jj * P : (jj + 1) * P],
                            vb[i // 4][:, i % 4, :],
                            start=(i == 0),
                            stop=(i == NKC - 1),
                        )
                    osb = o_pool.tile([P, D], F32, name="osb")
                    recip = _CACHE["recips"][(b, j)]
                    nc.scalar.activation(osb[:], op[:], AF.Copy, scale=recip[:])
                    nc.scalar.dma_start(o_d[b, j * P : (j + 1) * P, :], osb[:])

    _CACHE.pop("recips", None)
    nc.compile()
    return nc


def _get_program():
    if "nc" not in _CACHE:
        _CACHE["nc"] = _build_program()
    return _CACHE["nc"]


def kernel(query, keys, values, mask, _trace=False):
    nc = _get_program()
    query = np.ascontiguousarray(query, dtype=np.float32)
    keys = np.ascontiguousarray(keys, dtype=np.float32)
    values = np.ascontiguousarray(values, dtype=np.float32)
    maskbias = np.where(mask, np.float32(MASK_BIAS), np.float32(0.0))

    in_maps = []
    for i in range(NCORES):
        sl = slice(i * BPC, (i + 1) * BPC)
        in_maps.append(
            {
                "query": query[sl],
                "keys": keys[sl],
                "values": values[sl],
                "maskbias": maskbias[sl],
            }
        )

    res = run_bass_kernel_spmd(nc, in_maps, core_ids=list(range(NCORES)), trace=_trace)
    _CACHE["last_result"] = res

    out = np.concatenate([res.results[i]["out"] for i in range(NCORES)], axis=0)
    wts = np.concatenate([res.results[i]["weights"] for i in range(NCORES)], axis=0)
    return (out, wts)


# revision 38
# speedup vs baseline: 1.0224x; 1.0224x over previous
"""Batch-sharded scaled-dot-product attention (with weights output) on 8 TRN2 NeuronCores.

Problem: B=16, LQ=LK=2048, D=512, f32 inputs. Returns (weighted_sum, weights)
exactly like the reference (softmax over keys, bool mask with True=masked,
masked weights exactly 0).

Sharding: batch dim 16 -> 2 batches per core, fully data-parallel (no
collectives).

Per-core dataflow (no DMA transposes of E; S^T computed on the PE):
  - Q,K,V loaded f32 (plain SWDGE), cast to bf16 on DVE.
  - Q^T,K^T via xbar DMA transpose, 4 big calls each per batch, Sync queue
    only (xbar state is global; two queues corrupt).
  - S   = Q^T.T @ K^T  (+ ones x bias_row matmul adds -1e9 to masked cols)
    -> exp with accum_out rowsums -> recip -> W=E*recip -> one store/q-tile.
  - S^T = K^T.T @ Q^T (same operands swapped); exp' applies the mask via
    ACT per-partition bias (-1e9 on masked key rows) -> E^T bf16 tiles.
    Query dim processed in 2 halves to bound SBUF.
  - O   = sum_i E^T_i.T @ V_i, scaled by recip during the PSUM->SBUF copy.
"""

import math
import numpy as np

from concourse import mybir, bacc, tile
from concourse.bass_utils import run_bass_kernel_spmd

F32 = mybir.dt.float32
BF16 = mybir.dt.bfloat16
AF = mybir.ActivationFunctionType

B, LQ, LK, D = 16, 2048, 2048, 512
NCORES = 8
BPC = B // NCORES  # batches per core
P = 128
DC = D // P        # 4 contraction chunks
NKC = LK // P      # 16 key chunks
NQT = LQ // P      # 16 query tiles
SEG = 512
NSEG = LK // SEG   # 4 key segments
NQH = NQT // 2     # q-tiles per half
SCALE = 1.0 / math.sqrt(D)
MASK_BIAS = -1.0e9

_CACHE = {}


def _build_program():
    from contextlib import ExitStack

    nc = bacc.Bacc("TRN2", target_bir_lowering=False, debug=False, num_devices=NCORES)
    q_d = nc.dram_tensor("query", [BPC, LQ, D], F32, kind="ExternalInput").ap()
    k_d = nc.dram_tensor("keys", [BPC, LK, D], F32, kind="ExternalInput").ap()
    v_d = nc.dram_tensor("values", [BPC, LK, D], F32, kind="ExternalInput").ap()
    mb_d = nc.dram_tensor("maskbias", [BPC, LK], F32, kind="ExternalInput").ap()
    o_d = nc.dram_tensor("out", [BPC, LQ, D], F32, kind="ExternalOutput").ap()
    w_d = nc.dram_tensor("weights", [BPC, LQ, LK], F32, kind="ExternalOutput").ap()

    with tile.TileContext(nc) as tc, ExitStack() as ctx:
        const_pool = ctx.enter_context(tc.tile_pool(name="const", bufs=1))
        bias_pool = ctx.enter_context(tc.tile_pool(name="bias", bufs=2))
        mbias_pool = ctx.enter_context(tc.tile_pool(name="mbias", bufs=2))
        stage_pool = ctx.enter_context(tc.tile_pool(name="stage", bufs=3))
        kload_pool = ctx.enter_context(tc.tile_pool(name="kload", bufs=2))
        qload_pool = ctx.enter_context(tc.tile_pool(name="qload", bufs=2))
        v_pool = ctx.enter_context(tc.tile_pool(name="v", bufs=8))
        kt_pool = ctx.enter_context(tc.tile_pool(name="kt", bufs=2))
        qt_pool = ctx.enter_context(tc.tile_pool(name="qt", bufs=2))
        e_pool = ctx.enter_context(tc.tile_pool(name="e", bufs=8))
        et_pool = ctx.enter_context(tc.tile_pool(name="et", bufs=16))
        w_pool = ctx.enter_context(tc.tile_pool(name="w", bufs=2))
        o_pool = ctx.enter_context(tc.tile_pool(name="o", bufs=2))
        stat_pool = ctx.enter_context(tc.tile_pool(name="stat", bufs=4))
        recip_pool = ctx.enter_context(tc.tile_pool(name="recip", bufs=20))
        s_psum = ctx.enter_context(tc.tile_pool(name="spsum", bufs=3, space="PSUM"))
        s2_psum = ctx.enter_context(tc.tile_pool(name="s2psum", bufs=3, space="PSUM"))
        o_psum = ctx.enter_context(tc.tile_pool(name="opsum", bufs=2, space="PSUM"))

        ones = const_pool.tile([1, P], BF16)
        nc.gpsimd.memset(ones[:], 1.0)

        for b in range(BPC):
            # ---------------- per-batch prep ----------------
            bias_row = bias_pool.tile([1, LK], BF16)
            nc.gpsimd.dma_start(bias_row[:], mb_d[b : b + 1, :])  # f32 -> bf16

            # mask bias as per-partition columns: mbias[:, i] = bias_row[128i..]
            pm = s2_psum.tile([P, SEG], F32, name="s2")
            for i in range(NKC):
                nc.tensor.matmul(
                    pm[:, i : i + 1],
                    bias_row[:, i * P : (i + 1) * P],
                    ones[:, 0:1],
                    start=True,
                    stop=True,
                )
            mbias = mbias_pool.tile([P, NKC], F32)
            nc.vector.tensor_copy(mbias[:], pm[:, :NKC])

            k_r = k_d[b].rearrange("(n p) d -> p n d", p=P)  # [128, NKC, D]
            v_r = v_d[b].rearrange("(n p) d -> p n d", p=P)
            q_r = q_d[b].rearrange("(n p) d -> p n d", p=P)

            # kt/qt layout: [128(d-local), chunk i/j, d-chunk c, 128]
            kt = kt_pool.tile([P, NKC, DC, P], BF16, name="kt")
            qt = qt_pool.tile([P, NQT, DC, P], BF16, name="qt")
            vb = []
            if b == 0:
                # Cold start: nothing to hide the prep under. K first (the
                # first q-tile consumes all of K^T), Q concurrently on the
                # scalar HWDGE queue (idle this early; per-queue streams are
                # independent), V last (mm2 needs it much later and the
                # 16-deep et pool rides out the wait).
                for g in range(4):
                    ks = stage_pool.tile([P, 4, D], F32, name="stage")
                    nc.gpsimd.dma_start(ks[:], k_r[:, 4 * g : 4 * g + 4, :])
                    kg = kload_pool.tile([P, 4, D], BF16, name="kg")
                    nc.vector.tensor_copy(kg[:], ks[:])
                    nc.sync.dma_start(
                        kt[:, 4 * g : 4 * g + 4, :, :], kg[:], transpose=True
                    )
                    qs = stage_pool.tile([P, 4, D], F32, name="stage")
                    nc.scalar.dma_start(qs[:], q_r[:, 4 * g : 4 * g + 4, :])
                    qg = qload_pool.tile([P, 4, D], BF16, name="qg")
                    nc.vector.tensor_copy(qg[:], qs[:])
                    nc.sync.dma_start(
                        qt[:, 4 * g : 4 * g + 4, :, :], qg[:], transpose=True
                    )
                for g in range(4):
                    vs = stage_pool.tile([P, 4, D], F32, name="stage")
                    nc.gpsimd.dma_start(vs[:], v_r[:, 4 * g : 4 * g + 4, :])
                    vt = v_pool.tile([P, 4, D], BF16, name="vt")
                    nc.vector.tensor_copy(vt[:], vs[:])
                    vb.append(vt)
            else:
                for g in range(4):
                    ks = stage_pool.tile([P, 4, D], F32, name="stage")
                    nc.gpsimd.dma_start(ks[:], k_r[:, 4 * g : 4 * g + 4, :])
                    kg = kload_pool.tile([P, 4, D], BF16, name="kg")
                    nc.vector.tensor_copy(kg[:], ks[:])
                    nc.sync.dma_start(
                        kt[:, 4 * g : 4 * g + 4, :, :], kg[:], transpose=True
                    )
                    qs = stage_pool.tile([P, 4, D], F32, name="stage")
                    nc.gpsimd.dma_start(qs[:], q_r[:, 4 * g : 4 * g + 4, :])
                    qg = qload_pool.tile([P, 4, D], BF16, name="qg")
                    nc.vector.tensor_copy(qg[:], qs[:])
                    nc.sync.dma_start(
                        qt[:, 4 * g : 4 * g + 4, :, :], qg[:], transpose=True
                    )
                    vs = stage_pool.tile([P, 4, D], F32, name="stage")
                    nc.gpsimd.dma_start(vs[:], v_r[:, 4 * g : 4 * g + 4, :])
                    vt = v_pool.tile([P, 4, D], BF16, name="vt")
                    nc.vector.tensor_copy(vt[:], vs[:])
                    vb.append(vt)

            # ---------------- halves of the query dim ----------------
            for h in range(2):
                jbase = h * NQH
                # -- W side: S, exp, rowsums, W out --
                for jj in range(NQH):
                    j = jbase + jj
                    rowsums = stat_pool.tile([P, NSEG], F32, name="rowsums")
                    esegs = []
                    for s in range(NSEG):
                        sp = s_psum.tile([P, SEG], F32, name="sp")
                        for c in range(DC):
                            nc.tensor.matmul(
                                sp[:],
                                qt[:, j, c, :],
                                kt[:, 4 * s : 4 * s + 4, c, :],
                                start=(c == 0),
                                stop=False,
                            )
                        nc.tensor.matmul(
                            sp[:],
                            ones[:],
                            bias_row[:, s * SEG : (s + 1) * SEG],
                            start=False,
                            stop=True,
                        )
                        e = e_pool.tile([P, SEG], BF16, name="e")
                        nc.scalar.activation(
                            e[:],
                            sp[:],
                            AF.Exp,
                            scale=SCALE,
                            accum_out=rowsums[:, s : s + 1],
                        )
                        esegs.append(e)

                    sum_all = stat_pool.tile([P, 1], F32, name="sumall")
                    nc.vector.reduce_sum(
                        sum_all[:], rowsums[:], axis=mybir.AxisListType.X
                    )
                    recip = recip_pool.tile([P, 1], F32, name="recip")
                    nc.vector.reciprocal(recip[:], sum_all[:])
                    _CACHE.setdefault("recips", {})[(b, j)] = recip

                    for wh in range(2):
                        wf = w_pool.tile([P, LK // 2], F32, name="wf")
                        for s2i in range(2):
                            s = 2 * wh + s2i
                            nc.vector.tensor_scalar_mul(
                                wf[:, s2i * SEG : (s2i + 1) * SEG],
                                esegs[s][:],
                                recip[:],
                            )
                        nc.scalar.dma_start(
                            w_d[
                                b,
                                j * P : (j + 1) * P,
                                wh * (LK // 2) : (wh + 1) * (LK // 2),
                            ],
                            wf[:],
                        )

                # -- S^T side: E^T tiles for this half --
                ets = []
                for i in range(NKC):
                    et = et_pool.tile([P, NQH * P], BF16, name="et")
                    for t in range(2):
                        s2 = s2_psum.tile([P, SEG], F32, name="s2")
                        tq = 2 * h + t
                        for c in range(DC):
                            nc.tensor.matmul(
                                s2[:],
                                kt[:, i, c, :],
                                qt[:, 4 * tq : 4 * tq + 4, c, :],
                                start=(c == 0),
                                stop=(c == DC - 1),
                            )
                        nc.scalar.activation(
                            et[:, t * SEG : (t + 1) * SEG],
                            s2[:],
                            AF.Exp,
                            scale=SCALE,
                            bias=mbias[:, i : i + 1],
                        )
                    ets.append(et)

                # -- O side: mm2 + scale + store --
                for jj in range(NQH):
                    j = jbase + jj
                    op = o_psum.tile([P, D], F32, name="op")
                    for i in range(NKC):
                        nc.tensor.matmul(
                            op[:],
                            ets[i][:, jj * P : (jj + 1) * P],
                            vb[i // 4][:, i % 4, :],
                            start=(i == 0),
                            stop=(i == NKC - 1),
                        )
                    osb = o_pool.tile([P, D], F32, name="osb")
                    recip = _CACHE["recips"][(b, j)]
                    nc.vector.tensor_scalar_mul(osb[:], op[:], recip[:])
                    nc.scalar.dma_start(o_d[b, j * P : (j + 1) * P, :], osb[:])

    _CACHE.pop("recips", None)
    nc.compile()
    return nc


def _get_program():
    if "nc" not in _CACHE:
        _CACHE["nc"] = _build_program()
    return _CACHE["nc"]


def kernel(query, keys, values, mask, _trace=False):
    nc = _get_program()
    query = np.ascontiguousarray(query, dtype=np.float32)
    keys = np.ascontiguousarray(keys, dtype=np.float32)
    values = np.ascontiguousarray(values, dtype=np.float32)
    maskbias = np.where(mask, np.float32(MASK_BIAS), np.float32(0.0))

    in_maps = []
    for i in range(NCORES):
        sl = slice(i * BPC, (i + 1) * BPC)
        in_maps.append(
            {
                "query": query[sl],
                "keys": keys[sl],
                "values": values[sl],
                "maskbias": maskbias[sl],
            }
        )

    res = run_bass_kernel_spmd(nc, in_maps, core_ids=list(range(NCORES)), trace=_trace)
    _CACHE["last_result"] = res

    out = np.concatenate([res.results[i]["out"] for i in range(NCORES)], axis=0)
    wts = np.concatenate([res.results[i]["weights"] for i in range(NCORES)], axis=0)
    return (out, wts)


# revision 39
# speedup vs baseline: 1.0318x; 1.0092x over previous
"""Batch-sharded scaled-dot-product attention (with weights output) on 8 TRN2 NeuronCores.

Problem: B=16, LQ=LK=2048, D=512, f32 inputs. Returns (weighted_sum, weights)
exactly like the reference (softmax over keys, bool mask with True=masked,
masked weights exactly 0).

Sharding: batch dim 16 -> 2 batches per core, fully data-parallel (no
collectives).

Per-core dataflow (no DMA transposes of E; S^T computed on the PE):
  - Q,K,V loaded f32 (plain SWDGE), cast to bf16 on DVE.
  - Q^T,K^T via xbar DMA transpose, 4 big calls each per batch, Sync queue
    only (xbar state is global; two queues corrupt).
  - S   = Q^T.T @ K^T  (+ ones x bias_row matmul adds -1e9 to masked cols)
    -> exp with accum_out rowsums -> recip -> W=E*recip -> one store/q-tile.
  - S^T = K^T.T @ Q^T (same operands swapped); exp' applies the mask via
    ACT per-partition bias (-1e9 on masked key rows) -> E^T bf16 tiles.
    Query dim processed in 2 halves to bound SBUF.
  - O   = sum_i E^T_i.T @ V_i, scaled by recip during the PSUM->SBUF copy.
"""

import math
import numpy as np

from concourse import mybir, bacc, tile
from concourse.bass_utils import run_bass_kernel_spmd

F32 = mybir.dt.float32
BF16 = mybir.dt.bfloat16
AF = mybir.ActivationFunctionType

B, LQ, LK, D = 16, 2048, 2048, 512
NCORES = 8
BPC = B // NCORES  # batches per core
P = 128
DC = D // P        # 4 contraction chunks
NKC = LK // P      # 16 key chunks
NQT = LQ // P      # 16 query tiles
SEG = 512
NSEG = LK // SEG   # 4 key segments
NQH = NQT // 2     # q-tiles per half
SCALE = 1.0 / math.sqrt(D)
MASK_BIAS = -1.0e9

_CACHE = {}


def _build_program():
    from contextlib import ExitStack

    nc = bacc.Bacc("TRN2", target_bir_lowering=False, debug=False, num_devices=NCORES)
    q_d = nc.dram_tensor("query", [BPC, LQ, D], F32, kind="ExternalInput").ap()
    k_d = nc.dram_tensor("keys", [BPC, LK, D], F32, kind="ExternalInput").ap()
    v_d = nc.dram_tensor("values", [BPC, LK, D], F32, kind="ExternalInput").ap()
    mb_d = nc.dram_tensor("maskbias", [BPC, LK], F32, kind="ExternalInput").ap()
    o_d = nc.dram_tensor("out", [BPC, LQ, D], F32, kind="ExternalOutput").ap()
    w_d = nc.dram_tensor("weights", [BPC, LQ, LK], F32, kind="ExternalOutput").ap()

    with tile.TileContext(nc) as tc, ExitStack() as ctx:
        const_pool = ctx.enter_context(tc.tile_pool(name="const", bufs=1))
        bias_pool = ctx.enter_context(tc.tile_pool(name="bias", bufs=2))
        mbias_pool = ctx.enter_context(tc.tile_pool(name="mbias", bufs=2))
        stage_pool = ctx.enter_context(tc.tile_pool(name="stage", bufs=3))
        kload_pool = ctx.enter_context(tc.tile_pool(name="kload", bufs=2))
        qload_pool = ctx.enter_context(tc.tile_pool(name="qload", bufs=2))
        v_pool = ctx.enter_context(tc.tile_pool(name="v", bufs=8))
        kt_pool = ctx.enter_context(tc.tile_pool(name="kt", bufs=2))
        qt_pool = ctx.enter_context(tc.tile_pool(name="qt", bufs=2))
        e_pool = ctx.enter_context(tc.tile_pool(name="e", bufs=8))
        et_pool = ctx.enter_context(tc.tile_pool(name="et", bufs=16))
        w_pool = ctx.enter_context(tc.tile_pool(name="w", bufs=2))
        o_pool = ctx.enter_context(tc.tile_pool(name="o", bufs=2))
        stat_pool = ctx.enter_context(tc.tile_pool(name="stat", bufs=4))
        recip_pool = ctx.enter_context(tc.tile_pool(name="recip", bufs=20))
        s_psum = ctx.enter_context(tc.tile_pool(name="spsum", bufs=3, space="PSUM"))
        s2_psum = ctx.enter_context(tc.tile_pool(name="s2psum", bufs=3, space="PSUM"))
        o_psum = ctx.enter_context(tc.tile_pool(name="opsum", bufs=2, space="PSUM"))

        ones = const_pool.tile([1, P], BF16)
        nc.gpsimd.memset(ones[:], 1.0)

        def emit_prep(b):
            # ---------------- per-batch prep ----------------
            bias_row = bias_pool.tile([1, LK], BF16)
            nc.gpsimd.dma_start(bias_row[:], mb_d[b : b + 1, :])  # f32 -> bf16

            # mask bias as per-partition columns: mbias[:, i] = bias_row[128i..]
            pm = s2_psum.tile([P, SEG], F32, name="s2")
            for i in range(NKC):
                nc.tensor.matmul(
                    pm[:, i : i + 1],
                    bias_row[:, i * P : (i + 1) * P],
                    ones[:, 0:1],
                    start=True,
                    stop=True,
                )
            mbias = mbias_pool.tile([P, NKC], F32)
            nc.vector.tensor_copy(mbias[:], pm[:, :NKC])

            k_r = k_d[b].rearrange("(n p) d -> p n d", p=P)  # [128, NKC, D]
            v_r = v_d[b].rearrange("(n p) d -> p n d", p=P)
            q_r = q_d[b].rearrange("(n p) d -> p n d", p=P)

            # kt/qt layout: [128(d-local), chunk i/j, d-chunk c, 128]
            kt = kt_pool.tile([P, NKC, DC, P], BF16, name="kt")
            qt = qt_pool.tile([P, NQT, DC, P], BF16, name="qt")
            vb = []
            if b == 0:
                # Cold start: nothing to hide the prep under. K first (the
                # first q-tile consumes all of K^T), Q concurrently on the
                # scalar HWDGE queue (idle this early; per-queue streams are
                # independent), V last (mm2 needs it much later and the
                # 16-deep et pool rides out the wait).
                for g in range(4):
                    ks = stage_pool.tile([P, 4, D], F32, name="stage")
                    nc.gpsimd.dma_start(ks[:], k_r[:, 4 * g : 4 * g + 4, :])
                    kg = kload_pool.tile([P, 4, D], BF16, name="kg")
                    nc.vector.tensor_copy(kg[:], ks[:])
                    nc.sync.dma_start(
                        kt[:, 4 * g : 4 * g + 4, :, :], kg[:], transpose=True
                    )
                    qs = stage_pool.tile([P, 4, D], F32, name="stage")
                    nc.scalar.dma_start(qs[:], q_r[:, 4 * g : 4 * g + 4, :])
                    qg = qload_pool.tile([P, 4, D], BF16, name="qg")
                    nc.vector.tensor_copy(qg[:], qs[:])
                    nc.sync.dma_start(
                        qt[:, 4 * g : 4 * g + 4, :, :], qg[:], transpose=True
                    )
                for g in range(4):
                    vs = stage_pool.tile([P, 4, D], F32, name="stage")
                    nc.gpsimd.dma_start(vs[:], v_r[:, 4 * g : 4 * g + 4, :])
                    vt = v_pool.tile([P, 4, D], BF16, name="vt")
                    nc.vector.tensor_copy(vt[:], vs[:])
                    vb.append(vt)
            else:
                for g in range(4):
                    ks = stage_pool.tile([P, 4, D], F32, name="stage")
                    nc.gpsimd.dma_start(ks[:], k_r[:, 4 * g : 4 * g + 4, :])
                    kg = kload_pool.tile([P, 4, D], BF16, name="kg")
                    nc.vector.tensor_copy(kg[:], ks[:])
                    nc.sync.dma_start(
                        kt[:, 4 * g : 4 * g + 4, :, :], kg[:], transpose=True
                    )
                    qs = stage_pool.tile([P, 4, D], F32, name="stage")
                    nc.gpsimd.dma_start(qs[:], q_r[:, 4 * g : 4 * g + 4, :])
                    qg = qload_pool.tile([P, 4, D], BF16, name="qg")
                    nc.vector.tensor_copy(qg[:], qs[:])
                    nc.sync.dma_start(
                        qt[:, 4 * g : 4 * g + 4, :, :], qg[:], transpose=True
                    )
                    vs = stage_pool.tile([P, 4, D], F32, name="stage")
                    nc.gpsimd.dma_start(vs[:], v_r[:, 4 * g : 4 * g + 4, :])
                    vt = v_pool.tile([P, 4, D], BF16, name="vt")
                    nc.vector.tensor_copy(vt[:], vs[:])
                    vb.append(vt)
            return bias_row, mbias, kt, qt, vb

        preps = {0: emit_prep(0)}
        for b in range(BPC):
            bias_row, mbias, kt, qt, vb = preps.pop(b)
            # ---------------- halves of the query dim ----------------
            for h in range(2):
                if h == 1 and b + 1 < BPC:
                    # emit the next batch's prep here so its priority sits
                    # below this batch's first half: loads/casts/transposes
                    # overlap the h=1 compute instead of trailing it.
                    preps[b + 1] = emit_prep(b + 1)
                jbase = h * NQH
                # -- W side: S, exp, rowsums, W out --
                for jj in range(NQH):
                    j = jbase + jj
                    rowsums = stat_pool.tile([P, NSEG], F32, name="rowsums")
                    esegs = []
                    for s in range(NSEG):
                        sp = s_psum.tile([P, SEG], F32, name="sp")
                        for c in range(DC):
                            nc.tensor.matmul(
                                sp[:],
                                qt[:, j, c, :],
                                kt[:, 4 * s : 4 * s + 4, c, :],
                                start=(c == 0),
                                stop=False,
                            )
                        nc.tensor.matmul(
                            sp[:],
                            ones[:],
                            bias_row[:, s * SEG : (s + 1) * SEG],
                            start=False,
                            stop=True,
                        )
                        e = e_pool.tile([P, SEG], BF16, name="e")
                        nc.scalar.activation(
                            e[:],
                            sp[:],
                            AF.Exp,
                            scale=SCALE,
                            accum_out=rowsums[:, s : s + 1],
                        )
                        esegs.append(e)

                    sum_all = stat_pool.tile([P, 1], F32, name="sumall")
                    nc.vector.reduce_sum(
                        sum_all[:], rowsums[:], axis=mybir.AxisListType.X
                    )
                    recip = recip_pool.tile([P, 1], F32, name="recip")
                    nc.vector.reciprocal(recip[:], sum_all[:])
                    _CACHE.setdefault("recips", {})[(b, j)] = recip

                    for wh in range(2):
                        wf = w_pool.tile([P, LK // 2], F32, name="wf")
                        for s2i in range(2):
                            s = 2 * wh + s2i
                            nc.vector.tensor_scalar_mul(
                                wf[:, s2i * SEG : (s2i + 1) * SEG],
                                esegs[s][:],
                                recip[:],
                            )
                        nc.scalar.dma_start(
                            w_d[
                                b,
                                j * P : (j + 1) * P,
                                wh * (LK // 2) : (wh + 1) * (LK // 2),
                            ],
                            wf[:],
                        )

                # -- S^T side: E^T tiles for this half --
                ets = []
                for i in range(NKC):
                    et = et_pool.tile([P, NQH * P], BF16, name="et")
                    for t in range(2):
                        s2 = s2_psum.tile([P, SEG], F32, name="s2")
                        tq = 2 * h + t
                        for c in range(DC):
                            nc.tensor.matmul(
                                s2[:],
                                kt[:, i, c, :],
                                qt[:, 4 * tq : 4 * tq + 4, c, :],
                                start=(c == 0),
                                stop=(c == DC - 1),
                            )
                        nc.scalar.activation(
                            et[:, t * SEG : (t + 1) * SEG],
                            s2[:],
                            AF.Exp,
                            scale=SCALE,
                            bias=mbias[:, i : i + 1],
                        )
                    ets.append(et)

                # -- O side: mm2 + scale + store --
                for jj in range(NQH):
                    j = jbase + jj
                    op = o_psum.tile([P, D], F32, name="op")
                    for i in range(NKC):
                        nc.tensor.matmul(
                            op[:],
                            ets[i][:, jj * P : (jj + 1) * P],
                            vb[i // 4][:, i % 4, :],
                            start=(i == 0),
                            stop=(i == NKC - 1),
                        )
                    osb = o_pool.tile([P, D], F32, name="osb")
                    recip = _CACHE["recips"][(b, j)]
                    nc.scalar.activation(osb[:], op[:], AF.Copy, scale=recip[:])
                    nc.scalar.dma_start(o_d[b, j * P : (j + 1) * P, :], osb[:])

    _CACHE.pop("recips", None)
    nc.compile()
    return nc


def _get_program():
    if "nc" not in _CACHE:
        _CACHE["nc"] = _build_program()
    return _CACHE["nc"]


def kernel(query, keys, values, mask, _trace=False):
    nc = _get_program()
    query = np.ascontiguousarray(query, dtype=np.float32)
    keys = np.ascontiguousarray(keys, dtype=np.float32)
    values = np.ascontiguousarray(values, dtype=np.float32)
    maskbias = np.where(mask, np.float32(MASK_BIAS), np.float32(0.0))

    in_maps = []
    for i in range(NCORES):
        sl = slice(i * BPC, (i + 1) * BPC)
        in_maps.append(
            {
                "query": query[sl],
                "keys": keys[sl],
                "values": values[sl],
                "maskbias": maskbias[sl],
            }
        )

    res = run_bass_kernel_spmd(nc, in_maps, core_ids=list(range(NCORES)), trace=_trace)
    _CACHE["last_result"] = res

    out = np.concatenate([res.results[i]["out"] for i in range(NCORES)], axis=0)
    wts = np.concatenate([res.results[i]["weights"] for i in range(NCORES)], axis=0)
    return (out, wts)
